# revision 18
# baseline (speedup 1.0000x reference)
"""BatchedSharedLoRA TRN2 kernel, v3.

Math (per adapter a):  out[a] = x + SCALING * u / (||u||_rows + EPS),
where u = (x @ A_a) @ B_a,  x:[M,H], A:[H,R], B:[R,H].

Sharding: DATA-parallel over rows -- core i owns rows [i*512, (i+1)*512) of
the flattened x [4096, 4096] and computes all 8 adapters for its slice.

v3 changes vs v2 (295 us, PE HAM-cold + DVE 1x stt):
  * mm1 for pair p+2 runs as a DENSE 32-matmul block inside pair p's body
    (two-pair pipeline distance). Dense blocks keep the PE HAM-warm and the
    2-pair lag hides the ACT/DVE queue backlog that was stalling the PE at
    every pair boundary.
  * Residual add uses nc.vector.tensor_add (InstTensorTensor, 2x_1p for
    fp16 SBUF) instead of scalar_tensor_tensor (1x-only).
  * Residual work is split three ways to balance engines:
      'A' units: ACT evac (v = s*u, fp16) + DVE tensor_add (v + x)
      'B' units: DVE scalar_tensor_tensor fused (u*s + x) straight from PSUM
      'G' units: ACT evac + GPSIMD tensor_add
  * fp16 output, host-side x transpose/casts as in v2.

Per-core HBM traffic: 4 (x fp16) + 4 (xT bf16) + 4 (A) + 4 (B) + 32 (out
fp16) ~= 48 MiB -> ~140 us roofline at 358 GB/s.
"""

import numpy as np
import ml_dtypes

import concourse.bass as bass
import concourse.mybir as mybir
import concourse.tile as tile
from concourse import bacc, bass_utils

NADAPT = 8
BATCH, SEQ, H, R = 2, 2048, 4096, 64
M = BATCH * SEQ  # 4096
SCALING = 2.0
EPS = 1e-8

F32 = mybir.dt.float32
BF16 = mybir.dt.bfloat16
FP16 = mybir.dt.float16

MROWS = M // 8  # 512 rows per core
NBLK = MROWS // 128  # 4 m-blocks per core
KH = H // 128  # 32 contraction chunks for mm1
NPAIR = NADAPT // 2  # 4 adapter pairs

# Per-pair residual-unit engine pattern, indexed by (j*2 + e).
#   A: ACT evac + DVE tensor_add;  B: DVE fused from PSUM.
# GPSIMD adds were tried and removed: a GpSimd TENSOR_TENSOR running
# concurrently with a DVE TENSOR_TENSOR on the same x tile serializes the
# DVE op 4x (SBUF interference) and stalls the PE into HAM-cold.
# Each j gets at most one B so the two chunk-evacs of an iter can run on
# ACT and DVE in parallel.
UNIT_PATTERN = {
    0: "ABAAABAA",  # j0 (A,B), j1 (A,A), j2 (A,B), j3 (A,A)
    1: "AAABAAAB",  # j0 (A,A), j1 (A,B), j2 (A,A), j3 (A,B)
}


def build_kernel() -> bass.Bass:
    nc = bacc.Bacc(trn_type="TRN2")
    xr_d = nc.dram_tensor("xr", [MROWS, H], FP16, kind="ExternalInput")
    xt_d = nc.dram_tensor("xt", [128, KH * MROWS], BF16, kind="ExternalInput")
    a2_d = nc.dram_tensor("a2", [NPAIR * 128, KH * 128], BF16, kind="ExternalInput")
    b2_d = nc.dram_tensor("b2", [NPAIR * 128, H], BF16, kind="ExternalInput")
    bbtI_d = nc.dram_tensor("bbtI", [NPAIR * 128, 256], BF16, kind="ExternalInput")
    out_d = nc.dram_tensor("out", [NADAPT * MROWS, H], FP16, kind="ExternalOutput")

    with tile.TileContext(nc) as tc:
        with (
            tc.tile_pool(name="xpool", bufs=NBLK) as xpool,
            tc.tile_pool(name="xtpool", bufs=NBLK) as xtpool,
            tc.tile_pool(name="a2_pool", bufs=4) as a2_pool,
            tc.tile_pool(name="b2_pool", bufs=2) as b2_pool,
            tc.tile_pool(name="bbtI_pool", bufs=4) as bbtI_pool,
            tc.tile_pool(name="tT2_sb_pool", bufs=4) as tT2_sb_pool,
            tc.tile_pool(name="t2_sb_pool", bufs=2) as t2_sb_pool,
            tc.tile_pool(name="junk_pool", bufs=2) as junk_pool,
            tc.tile_pool(name="stat_pool", bufs=4) as stat_pool,
            tc.tile_pool(name="v_pool", bufs=3) as v_pool,
            tc.tile_pool(name="out_pool", bufs=4) as out_pool,
            tc.tile_pool(name="tT2_ps_pool", bufs=3, space="PSUM") as tT2_ps_pool,
            tc.tile_pool(name="u_ps_pool", bufs=2, space="PSUM") as u_ps_pool,
            tc.tile_pool(name="gt_ps_pool", bufs=1, space="PSUM") as gt_ps_pool,
        ):
            x_tiles = [
                xpool.tile([128, H], FP16, name=f"x_sb_{j}", tag="x_sb")
                for j in range(NBLK)
            ]

            xt_tiles = [
                xtpool.tile([128, KH // NBLK, MROWS], BF16, name=f"xt_{g}", tag="xt")
                for g in range(NBLK)
            ]

            def load_a2(p):
                a2_sb = a2_pool.tile([128, KH, 128], BF16, name=f"a2_{p}", tag="a2")
                nc.sync.dma_start(
                    out=a2_sb,
                    in_=a2_d.ap()[p * 128 : (p + 1) * 128, :].rearrange(
                        "p (k r) -> p k r", r=128
                    ),
                )
                return a2_sb

            def load_b2(p):
                b2_sb = b2_pool.tile([128, H], BF16, name=f"b2_{p}", tag="b2")
                nc.sync.dma_start(out=b2_sb, in_=b2_d.ap()[p * 128 : (p + 1) * 128, :])
                return b2_sb

            def load_bbtI(p):
                bbtI_sb = bbtI_pool.tile([128, 256], BF16, name=f"bbtI_{p}", tag="bbtI")
                nc.sync.dma_start(
                    out=bbtI_sb, in_=bbtI_d.ap()[p * 128 : (p + 1) * 128, :]
                )
                return bbtI_sb

            def mm1_block(p, a2_sb):
                """Dense 32-matmul mm1 for pair p: tT2 = A2_p^T @ x^T."""
                tT2_ps = tT2_ps_pool.tile(
                    [128, MROWS], F32, name=f"tT2_ps_{p}", tag="tT2_ps"
                )
                for k in range(KH):
                    nc.tensor.matmul(
                        tT2_ps,
                        a2_sb[:, k, :],
                        xt_tiles[k // 8][:, k % 8, :],
                        start=(k == 0),
                        stop=(k == KH - 1),
                    )
                return tT2_ps

            def norm_chain(p, tT2_ps, bbtI_sb):
                """tT2 evac + row-norm scales s = 2/(||u||+EPS) for pair p."""
                tT2_bf = tT2_sb_pool.tile(
                    [128, MROWS], BF16, name=f"tT2_{p}", tag="tT2"
                )
                nc.scalar.copy(out=tT2_bf, in_=tT2_ps)
                t2_all = t2_sb_pool.tile(
                    [128, NBLK, 128], BF16, name=f"t2_{p}", tag="t2"
                )
                ssq8 = stat_pool.tile(
                    [128, 2 * NBLK], F32, name=f"ssq8_{p}", tag="ssq8"
                )
                for jh in range(2):  # two j-halves so gt fits one PSUM bank
                    gt_ps = gt_ps_pool.tile(
                        [128, 2, 256], F32, name=f"gt_ps_{p}_{jh}", tag="gt"
                    )
                    for jj in range(2):
                        j = jh * 2 + jj
                        nc.tensor.matmul(
                            gt_ps[:, jj, :],
                            tT2_bf[:, j * 128 : (j + 1) * 128],
                            bbtI_sb,
                            start=True,
                            stop=True,
                        )
                    nc.scalar.copy(
                        out=t2_all[:, jh * 2 : jh * 2 + 2, :],
                        in_=gt_ps[:, :, 128:256],
                    )
                    for jj in range(2):
                        for e in range(2):
                            j = jh * 2 + jj
                            junk = junk_pool.tile(
                                [128, R], BF16, name=f"junk_{p}_{j}_{e}", tag="junk"
                            )
                            c = j * 2 + e
                            nc.vector.affine_mul_reduce(
                                out=junk,
                                accum_out=ssq8[:, c : c + 1],
                                in0=gt_ps[:, jj, e * R : (e + 1) * R],
                                in1=t2_all[:, j, e * R : (e + 1) * R],
                                scale=1.0,
                                bias=0.0,
                            )
                # nh = 0.5*||u|| + 0.5*EPS;  s = 1/nh = 2/(||u||+EPS)
                nh8 = stat_pool.tile([128, 2 * NBLK], F32, name=f"nh8_{p}", tag="nh8")
                nc.scalar.activation(
                    out=nh8, in_=ssq8, func=mybir.ActivationFunctionType.Sqrt,
                    scale=0.25,
                )
                nc.vector.tensor_scalar_add(out=nh8, in0=nh8, scalar1=EPS * 0.5)
                s8 = stat_pool.tile([128, 2 * NBLK], F32, name=f"s8_{p}", tag="s8")
                nc.vector.reciprocal(out=s8, in_=nh8)
                return tT2_bf, s8

            def mm2_body(p, tT2_bf, s8, b2_sb, dma_ctr):
                """mm2 + residual + out-DMA for pair p."""
                pat = UNIT_PATTERN[p % 2]
                out_sbs = [None, None]
                v4s = [None, None]
                for it in range(16):
                    j, n = divmod(it, 4)
                    if n == 0:
                        for e in range(2):
                            a = 2 * p + e
                            out_sbs[e] = out_pool.tile(
                                [128, H], FP16, name=f"out_{a}_{j}", tag="out"
                            )
                            if pat[j * 2 + e] == "A":
                                v4s[e] = v_pool.tile(
                                    [128, H], FP16, name=f"v_{a}_{j}", tag="v"
                                )
                    u_ps = [None, None]
                    for e in range(2):
                        u_ps[e] = u_ps_pool.tile(
                            [128, 1024], F32, name=f"u_{p}_{it}_{e}", tag="u"
                        )
                    # pair matmuls: adapter a on PE rows 0-63, adapter b on
                    # rows 64-127. Same-weight MMs adjacent to cut LDW churn.
                    for e in range(2):
                        for half in range(2):
                            c0 = n * 1024 + half * 512
                            nc.tensor.matmul(
                                u_ps[e][:, half * 512 : (half + 1) * 512],
                                tT2_bf[e * 64 : (e + 1) * 64, j * 128 : (j + 1) * 128],
                                b2_sb[e * 64 : (e + 1) * 64, c0 : c0 + 512],
                                start=True,
                                stop=True,
                            )
                    # residual: out = s*u + x
                    for e in range(2):
                        c = j * 2 + e
                        kind = pat[c]
                        xj = x_tiles[j][:, n * 1024 : (n + 1) * 1024]
                        if kind == "A":
                            nc.scalar.mul(
                                out=v4s[e][:, n * 1024 : (n + 1) * 1024],
                                in_=u_ps[e],
                                mul=s8[:, c : c + 1],
                            )
                        else:
                            nc.vector.scalar_tensor_tensor(
                                out=out_sbs[e][:, n * 1024 : (n + 1) * 1024],
                                in0=u_ps[e],
                                scalar=s8[:, c : c + 1],
                                in1=xj,
                                op0=mybir.AluOpType.mult,
                                op1=mybir.AluOpType.add,
                            )
                    if n == 3:
                        for e in range(2):
                            kind = pat[j * 2 + e]
                            if kind == "A":
                                nc.vector.tensor_add(out_sbs[e], v4s[e], x_tiles[j])
                            r0 = (2 * p + e) * MROWS + j * 128
                            dma_ctr[0] += 1
                            nc.gpsimd.dma_start(
                                out=out_d.ap()[r0 : r0 + 128, :], in_=out_sbs[e]
                            )

            # ---- Prologue: input DMAs; mm1 blocks for pairs 0-2 run
            # back-to-back EARLY so they execute in the initial HAM-warm
            # window (the PE gets activity-throttled to K=4/8 after ~75us of
            # load and never recovers; front-load the PE-dense work).
            a2_sbs = {0: load_a2(0)}
            for g in range(NBLK):
                nc.sync.dma_start(
                    out=xt_tiles[g],
                    in_=xt_d.ap()[
                        :, g * (KH // NBLK) * MROWS : (g + 1) * (KH // NBLK) * MROWS
                    ].rearrange("p (k m) -> p k m", m=MROWS),
                )
            a2_sbs[1] = load_a2(1)
            bbtI_sbs = {0: load_bbtI(0), 1: load_bbtI(1), 2: load_bbtI(2)}
            b2_sbs = {0: load_b2(0)}
            for j in range(NBLK):
                nc.sync.dma_start(
                    out=x_tiles[j], in_=xr_d.ap()[j * 128 : (j + 1) * 128, :]
                )
            a2_sbs[2] = load_a2(2)
            a2_sbs[3] = load_a2(3)
            bbtI_sbs[3] = load_bbtI(3)

            # mm1(0..2) back-to-back; mm1(3) takes pair-0's PSUM slot right
            # after norm_chain(0) evacuates it. All norm chains follow.
            tT2_pss, tT2_bfs, s8s = {}, {}, {}
            for q in (0, 1, 2):
                tT2_pss[q] = mm1_block(q, a2_sbs[q])
            tT2_bfs[0], s8s[0] = norm_chain(0, tT2_pss[0], bbtI_sbs[0])
            tT2_pss[3] = mm1_block(3, a2_sbs[3])
            for q in (1, 2, 3):
                tT2_bfs[q], s8s[q] = norm_chain(q, tT2_pss[q], bbtI_sbs[q])

            dma_ctr = [0]
            for p in range(NPAIR):
                if p + 1 < NPAIR:
                    b2_sbs[p + 1] = load_b2(p + 1)
                mm2_body(p, tT2_bfs[p], s8s[p], b2_sbs[p], dma_ctr)

    nc.compile()
    return nc


_NC_CACHE = {}


def _get_nc():
    if "nc" not in _NC_CACHE:
        _NC_CACHE["nc"] = build_kernel()
    return _NC_CACHE["nc"]


def _prep_inputs(x, lora_A, lora_B):
    xm = np.ascontiguousarray(np.asarray(x, dtype=np.float32)).reshape(M, H)
    lora_A = np.asarray(lora_A, dtype=np.float32)
    lora_B = np.asarray(lora_B, dtype=np.float32)
    assert lora_A.shape == (NADAPT, H, R) and lora_B.shape == (NADAPT, R, H)
    bf = ml_dtypes.bfloat16

    # A pairs: a2[pair*128 + p, k*128 + e*64 + r] = A[2*pair+e, k*128+p, r]
    a2 = np.ascontiguousarray(
        lora_A.astype(bf).reshape(NPAIR, 2, KH, 128, R).transpose(0, 3, 2, 1, 4)
    ).reshape(NPAIR * 128, KH * 128)
    # B pairs: b2[pair*128 + e*64 + r, h] = B[2*pair+e, r, h]
    b2 = np.ascontiguousarray(lora_B.astype(bf).reshape(NPAIR * 128, H))
    # BBT from the bf16-rounded B (consistent with mm2), block-diag per pair,
    # with an identity appended so one matmul yields both g = t@BBT and t.
    Bf = b2.astype(np.float32).reshape(NADAPT, R, H)
    bbt = np.einsum("arh,ash->ars", Bf, Bf)
    bbtI = np.zeros((NPAIR, 128, 256), np.float32)
    bbtI[:, 0:R, 0:R] = bbt[0::2]
    bbtI[:, R:128, R:128] = bbt[1::2]
    bbtI[:, :, 128:256] = np.eye(128, dtype=np.float32)[None]
    bbtI = np.ascontiguousarray(bbtI.astype(bf).reshape(NPAIR * 128, 256))

    x16 = xm.astype(np.float16)
    xtg = np.ascontiguousarray(xm.T).astype(bf)  # [H, M]
    return x16, xtg, a2, b2, bbtI


def run(inputs: dict, trace: bool = False):
    """Returns (output [8, 2, 2048, 4096] f32, BassKernelResults)."""
    x16, xtg, a2, b2, bbtI = _prep_inputs(
        inputs["x"], inputs["lora_A"], inputs["lora_B"]
    )

    nc = _get_nc()
    in_maps = []
    xtg_k = xtg.reshape(KH, 128, M)
    for i in range(8):
        xt_c = np.ascontiguousarray(
            xtg_k[:, :, i * MROWS : (i + 1) * MROWS].transpose(1, 0, 2)
        ).reshape(128, KH * MROWS)
        in_maps.append(
            {
                "xr": x16[i * MROWS : (i + 1) * MROWS],
                "xt": xt_c,
                "a2": a2,
                "b2": b2,
                "bbtI": bbtI,
            }
        )
    res = bass_utils.run_bass_kernel_spmd(
        nc, in_maps, core_ids=list(range(8)), trace=trace
    )
    # core i returns [NADAPT*MROWS, H] fp16 for its row slice; reassemble.
    parts = [r["out"].reshape(NADAPT, MROWS, H) for r in res.results]
    out = (
        np.concatenate(parts, axis=1).astype(np.float32).reshape(NADAPT, BATCH, SEQ, H)
    )
    return out, res


def kernel(x, lora_A, lora_B):
    out, _ = run({"x": x, "lora_A": lora_A, "lora_B": lora_B})
    return out


# revision 22
# speedup vs baseline: 1.4753x; 1.4753x over previous
"""BatchedSharedLoRA TRN2 kernel, v3.

Math (per adapter a):  out[a] = x + SCALING * u / (||u||_rows + EPS),
where u = (x @ A_a) @ B_a,  x:[M,H], A:[H,R], B:[R,H].

Sharding: DATA-parallel over rows -- core i owns rows [i*512, (i+1)*512) of
the flattened x [4096, 4096] and computes all 8 adapters for its slice.

v3 changes vs v2 (295 us, PE HAM-cold + DVE 1x stt):
  * mm1 for pair p+2 runs as a DENSE 32-matmul block inside pair p's body
    (two-pair pipeline distance). Dense blocks keep the PE HAM-warm and the
    2-pair lag hides the ACT/DVE queue backlog that was stalling the PE at
    every pair boundary.
  * Residual add uses nc.vector.tensor_add (InstTensorTensor, 2x_1p for
    fp16 SBUF) instead of scalar_tensor_tensor (1x-only).
  * Residual work is split three ways to balance engines:
      'A' units: ACT evac (v = s*u, fp16) + DVE tensor_add (v + x)
      'B' units: DVE scalar_tensor_tensor fused (u*s + x) straight from PSUM
      'G' units: ACT evac + GPSIMD tensor_add
  * fp16 output, host-side x transpose/casts as in v2.

Per-core HBM traffic: 4 (x fp16) + 4 (xT bf16) + 4 (A) + 4 (B) + 32 (out
fp16) ~= 48 MiB -> ~140 us roofline at 358 GB/s.
"""

import numpy as np
import ml_dtypes

import concourse.bass as bass
import concourse.mybir as mybir
import concourse.tile as tile
from concourse import bacc, bass_utils

NADAPT = 8
BATCH, SEQ, H, R = 2, 2048, 4096, 64
M = BATCH * SEQ  # 4096
SCALING = 2.0
EPS = 1e-8

F32 = mybir.dt.float32
BF16 = mybir.dt.bfloat16
FP16 = mybir.dt.float16

MROWS = M // 8  # 512 rows per core
NBLK = MROWS // 128  # 4 m-blocks per core
KH = H // 128  # 32 contraction chunks for mm1
NPAIR = NADAPT // 2  # 4 adapter pairs

# Per-pair residual-unit engine pattern, indexed by (j*2 + e).
#   A: ACT evac + DVE tensor_add;  B: DVE fused from PSUM.
# GPSIMD adds were tried and removed: a GpSimd TENSOR_TENSOR running
# concurrently with a DVE TENSOR_TENSOR on the same x tile serializes the
# DVE op 4x (SBUF interference) and stalls the PE into HAM-cold.
# Each j gets at most one B so the two chunk-evacs of an iter can run on
# ACT and DVE in parallel.
UNIT_PATTERN = {
    0: "ABAAABAB",  # j0 (A,B), j1 (A,A), j2 (A,B), j3 (A,B)
    1: "ABAAAAAB",  # j0 (A,B), j1 (A,A), j2 (A,A), j3 (A,B)
}


def build_kernel() -> bass.Bass:
    nc = bacc.Bacc(trn_type="TRN2")
    xr_d = nc.dram_tensor("xr", [MROWS, H], FP16, kind="ExternalInput")
    xt_d = nc.dram_tensor("xt", [128, KH * MROWS], BF16, kind="ExternalInput")
    a2_d = nc.dram_tensor("a2", [NPAIR * 128, KH * 128], BF16, kind="ExternalInput")
    b2_d = nc.dram_tensor("b2", [NPAIR * 128, H], BF16, kind="ExternalInput")
    bbtI_d = nc.dram_tensor("bbtI", [NPAIR * 128, 256], BF16, kind="ExternalInput")
    out_d = nc.dram_tensor("out", [NADAPT * MROWS, H], FP16, kind="ExternalOutput")

    with tile.TileContext(nc) as tc:
        with (
            tc.tile_pool(name="xpool", bufs=NBLK) as xpool,
            tc.tile_pool(name="xtpool", bufs=NBLK) as xtpool,
            tc.tile_pool(name="a2_pool", bufs=2) as a2_pool,
            tc.tile_pool(name="b2_pool", bufs=2) as b2_pool,
            tc.tile_pool(name="bbtI_pool", bufs=2) as bbtI_pool,
            tc.tile_pool(name="tT2_sb_pool", bufs=3) as tT2_sb_pool,
            tc.tile_pool(name="t2_sb_pool", bufs=2) as t2_sb_pool,
            tc.tile_pool(name="junk_pool", bufs=2) as junk_pool,
            tc.tile_pool(name="stat_pool", bufs=3) as stat_pool,
            tc.tile_pool(name="v_pool", bufs=3) as v_pool,
            tc.tile_pool(name="out_pool", bufs=4) as out_pool,
            tc.tile_pool(name="tT2_ps_pool", bufs=1, space="PSUM") as tT2_ps_pool,
            tc.tile_pool(name="u_ps_pool", bufs=3, space="PSUM") as u_ps_pool,
            tc.tile_pool(name="gt_ps_pool", bufs=1, space="PSUM") as gt_ps_pool,
        ):
            x_tiles = [
                xpool.tile([128, H], FP16, name=f"x_sb_{j}", tag="x_sb")
                for j in range(NBLK)
            ]

            xt_tiles = [
                xtpool.tile([128, KH // NBLK, MROWS], BF16, name=f"xt_{g}", tag="xt")
                for g in range(NBLK)
            ]

            def load_a2(p):
                a2_sb = a2_pool.tile([128, KH, 128], BF16, name=f"a2_{p}", tag="a2")
                nc.sync.dma_start(
                    out=a2_sb,
                    in_=a2_d.ap()[p * 128 : (p + 1) * 128, :].rearrange(
                        "p (k r) -> p k r", r=128
                    ),
                )
                return a2_sb

            def load_b2(p):
                b2_sb = b2_pool.tile([128, H], BF16, name=f"b2_{p}", tag="b2")
                nc.sync.dma_start(out=b2_sb, in_=b2_d.ap()[p * 128 : (p + 1) * 128, :])
                return b2_sb

            def load_bbtI(p):
                bbtI_sb = bbtI_pool.tile([128, 256], BF16, name=f"bbtI_{p}", tag="bbtI")
                nc.sync.dma_start(
                    out=bbtI_sb, in_=bbtI_d.ap()[p * 128 : (p + 1) * 128, :]
                )
                return bbtI_sb

            def mm1_block(p, a2_sb):
                """Dense 32-matmul mm1 for pair p: tT2 = A2_p^T @ x^T."""
                tT2_ps = tT2_ps_pool.tile(
                    [128, MROWS], F32, name=f"tT2_ps_{p}", tag="tT2_ps"
                )
                for k in range(KH):
                    nc.tensor.matmul(
                        tT2_ps,
                        a2_sb[:, k, :],
                        xt_tiles[k // 8][:, k % 8, :],
                        start=(k == 0),
                        stop=(k == KH - 1),
                    )
                return tT2_ps

            def norm_chain(p, tT2_ps, bbtI_sb):
                """tT2 evac + row-norm scales s = 2/(||u||+EPS) for pair p."""
                tT2_bf = tT2_sb_pool.tile(
                    [128, MROWS], BF16, name=f"tT2_{p}", tag="tT2"
                )
                nc.scalar.copy(out=tT2_bf, in_=tT2_ps)
                t2_all = t2_sb_pool.tile(
                    [128, NBLK, 128], BF16, name=f"t2_{p}", tag="t2"
                )
                ssq8 = stat_pool.tile(
                    [128, 2 * NBLK], F32, name=f"ssq8_{p}", tag="ssq8"
                )
                for jh in range(2):  # two j-halves so gt fits one PSUM bank
                    gt_ps = gt_ps_pool.tile(
                        [128, 2, 256], F32, name=f"gt_ps_{p}_{jh}", tag="gt"
                    )
                    for jj in range(2):
                        j = jh * 2 + jj
                        nc.tensor.matmul(
                            gt_ps[:, jj, :],
                            tT2_bf[:, j * 128 : (j + 1) * 128],
                            bbtI_sb,
                            start=True,
                            stop=True,
                        )
                    nc.scalar.copy(
                        out=t2_all[:, jh * 2 : jh * 2 + 2, :],
                        in_=gt_ps[:, :, 128:256],
                    )
                    for jj in range(2):
                        for e in range(2):
                            j = jh * 2 + jj
                            junk = junk_pool.tile(
                                [128, R], BF16, name=f"junk_{p}_{j}_{e}", tag="junk"
                            )
                            c = j * 2 + e
                            nc.vector.affine_mul_reduce(
                                out=junk,
                                accum_out=ssq8[:, c : c + 1],
                                in0=gt_ps[:, jj, e * R : (e + 1) * R],
                                in1=t2_all[:, j, e * R : (e + 1) * R],
                                scale=1.0,
                                bias=0.0,
                            )
                # nh = 0.5*||u|| + 0.5*EPS;  s = 1/nh = 2/(||u||+EPS)
                nh8 = stat_pool.tile([128, 2 * NBLK], F32, name=f"nh8_{p}", tag="nh8")
                nc.scalar.activation(
                    out=nh8, in_=ssq8, func=mybir.ActivationFunctionType.Sqrt,
                    scale=0.25,
                )
                nc.vector.tensor_scalar_add(out=nh8, in0=nh8, scalar1=EPS * 0.5)
                s8 = stat_pool.tile([128, 2 * NBLK], F32, name=f"s8_{p}", tag="s8")
                nc.vector.reciprocal(out=s8, in_=nh8)
                return tT2_bf, s8

            def mm2_body(p, tT2_bf, s8, b2_sb, dma_ctr):
                """mm2 + residual + out-DMA for pair p."""
                pat = UNIT_PATTERN[p % 2]
                out_sbs = [None, None]
                v4s = [None, None]
                for it in range(16):
                    j, n = divmod(it, 4)
                    if n == 0:
                        for e in range(2):
                            a = 2 * p + e
                            out_sbs[e] = out_pool.tile(
                                [128, H], FP16, name=f"out_{a}_{j}", tag="out"
                            )
                            if pat[j * 2 + e] == "A":
                                v4s[e] = v_pool.tile(
                                    [128, H], FP16, name=f"v_{a}_{j}", tag="v"
                                )
                    u_ps = [None, None]
                    for e in range(2):
                        u_ps[e] = u_ps_pool.tile(
                            [128, 1024], F32, name=f"u_{p}_{it}_{e}", tag="u"
                        )
                    # pair matmuls: adapter a on PE rows 0-63, adapter b on
                    # rows 64-127. Same-weight MMs adjacent to cut LDW churn.
                    for e in range(2):
                        for half in range(2):
                            c0 = n * 1024 + half * 512
                            nc.tensor.matmul(
                                u_ps[e][:, half * 512 : (half + 1) * 512],
                                tT2_bf[e * 64 : (e + 1) * 64, j * 128 : (j + 1) * 128],
                                b2_sb[e * 64 : (e + 1) * 64, c0 : c0 + 512],
                                start=True,
                                stop=True,
                            )
                    # residual: out = s*u + x
                    for e in range(2):
                        c = j * 2 + e
                        kind = pat[c]
                        xj = x_tiles[j][:, n * 1024 : (n + 1) * 1024]
                        if kind == "A":
                            nc.scalar.mul(
                                out=v4s[e][:, n * 1024 : (n + 1) * 1024],
                                in_=u_ps[e],
                                mul=s8[:, c : c + 1],
                            )
                        else:
                            nc.vector.scalar_tensor_tensor(
                                out=out_sbs[e][:, n * 1024 : (n + 1) * 1024],
                                in0=u_ps[e],
                                scalar=s8[:, c : c + 1],
                                in1=xj,
                                op0=mybir.AluOpType.mult,
                                op1=mybir.AluOpType.add,
                            )
                    if n == 3:
                        for e in range(2):
                            kind = pat[j * 2 + e]
                            if kind == "A":
                                nc.vector.tensor_add(out_sbs[e], v4s[e], x_tiles[j])
                            r0 = (2 * p + e) * MROWS + j * 128
                            dma_ctr[0] += 1
                            nc.gpsimd.dma_start(
                                out=out_d.ap()[r0 : r0 + 128, :], in_=out_sbs[e]
                            )

            # ---- Prologue: input DMAs; mm1+norms for pairs 0 and 1.
            a2_sbs = {0: load_a2(0)}
            for g in range(NBLK):
                nc.sync.dma_start(
                    out=xt_tiles[g],
                    in_=xt_d.ap()[
                        :, g * (KH // NBLK) * MROWS : (g + 1) * (KH // NBLK) * MROWS
                    ].rearrange("p (k m) -> p k m", m=MROWS),
                )
            a2_sbs[1] = load_a2(1)
            bbtI_sbs = {0: load_bbtI(0), 1: load_bbtI(1)}
            b2_sbs = {0: load_b2(0)}
            for j in range(NBLK):
                nc.sync.dma_start(
                    out=x_tiles[j], in_=xr_d.ap()[j * 128 : (j + 1) * 128, :]
                )

            tT2_bfs, s8s = {}, {}
            for q in (0, 1):
                tT2_ps = mm1_block(q, a2_sbs[q])
                tT2_bfs[q], s8s[q] = norm_chain(q, tT2_ps, bbtI_sbs[q])
            a2_sbs[2] = load_a2(2)
            bbtI_sbs[2] = load_bbtI(2)

            dma_ctr = [0]
            for p in range(NPAIR):
                if p + 1 < NPAIR:
                    b2_sbs[p + 1] = load_b2(p + 1)
                if p + 3 < NPAIR:
                    a2_sbs[p + 3] = load_a2(p + 3)
                    bbtI_sbs[p + 3] = load_bbtI(p + 3)
                if p + 2 < NPAIR:
                    tT2_ps = mm1_block(p + 2, a2_sbs[p + 2])
                    tT2_bfs[p + 2], s8s[p + 2] = norm_chain(
                        p + 2, tT2_ps, bbtI_sbs[p + 2]
                    )
                mm2_body(p, tT2_bfs[p], s8s[p], b2_sbs[p], dma_ctr)

    nc.compile()
    return nc


_NC_CACHE = {}


def _get_nc():
    if "nc" not in _NC_CACHE:
        _NC_CACHE["nc"] = build_kernel()
    return _NC_CACHE["nc"]


def _prep_inputs(x, lora_A, lora_B):
    xm = np.ascontiguousarray(np.asarray(x, dtype=np.float32)).reshape(M, H)
    lora_A = np.asarray(lora_A, dtype=np.float32)
    lora_B = np.asarray(lora_B, dtype=np.float32)
    assert lora_A.shape == (NADAPT, H, R) and lora_B.shape == (NADAPT, R, H)
    bf = ml_dtypes.bfloat16

    # A pairs: a2[pair*128 + p, k*128 + e*64 + r] = A[2*pair+e, k*128+p, r]
    a2 = np.ascontiguousarray(
        lora_A.astype(bf).reshape(NPAIR, 2, KH, 128, R).transpose(0, 3, 2, 1, 4)
    ).reshape(NPAIR * 128, KH * 128)
    # B pairs: b2[pair*128 + e*64 + r, h] = B[2*pair+e, r, h]
    b2 = np.ascontiguousarray(lora_B.astype(bf).reshape(NPAIR * 128, H))
    # BBT from the bf16-rounded B (consistent with mm2), block-diag per pair,
    # with an identity appended so one matmul yields both g = t@BBT and t.
    Bf = b2.astype(np.float32).reshape(NADAPT, R, H)
    bbt = np.einsum("arh,ash->ars", Bf, Bf)
    bbtI = np.zeros((NPAIR, 128, 256), np.float32)
    bbtI[:, 0:R, 0:R] = bbt[0::2]
    bbtI[:, R:128, R:128] = bbt[1::2]
    bbtI[:, :, 128:256] = np.eye(128, dtype=np.float32)[None]
    bbtI = np.ascontiguousarray(bbtI.astype(bf).reshape(NPAIR * 128, 256))

    x16 = xm.astype(np.float16)
    xtg = np.ascontiguousarray(xm.T).astype(bf)  # [H, M]
    return x16, xtg, a2, b2, bbtI


def run(inputs: dict, trace: bool = False):
    """Returns (output [8, 2, 2048, 4096] f32, BassKernelResults)."""
    x16, xtg, a2, b2, bbtI = _prep_inputs(
        inputs["x"], inputs["lora_A"], inputs["lora_B"]
    )

    nc = _get_nc()
    in_maps = []
    xtg_k = xtg.reshape(KH, 128, M)
    for i in range(8):
        xt_c = np.ascontiguousarray(
            xtg_k[:, :, i * MROWS : (i + 1) * MROWS].transpose(1, 0, 2)
        ).reshape(128, KH * MROWS)
        in_maps.append(
            {
                "xr": x16[i * MROWS : (i + 1) * MROWS],
                "xt": xt_c,
                "a2": a2,
                "b2": b2,
                "bbtI": bbtI,
            }
        )
    res = bass_utils.run_bass_kernel_spmd(
        nc, in_maps, core_ids=list(range(8)), trace=trace
    )
    # core i returns [NADAPT*MROWS, H] fp16 for its row slice; reassemble.
    parts = [r["out"].reshape(NADAPT, MROWS, H) for r in res.results]
    out = (
        np.concatenate(parts, axis=1).astype(np.float32).reshape(NADAPT, BATCH, SEQ, H)
    )
    return out, res


def kernel(x, lora_A, lora_B):
    out, _ = run({"x": x, "lora_A": lora_A, "lora_B": lora_B})
    return out
